# revision 19
# baseline (speedup 1.0000x reference)
"""Trainium2 Bass kernel for nn_Cluster_assigner (moe_routing).

Data-parallel over batch across 8 NeuronCores (4 batches/core):
  - x_embT[b] = W @ x[b] + bias  (d-major) via PE matmul
  - scores[b] = cluster_emb @ x_embT[b]         (c-major, raw attention scores)
  - x_emb[b]  = PE-transpose(x_embT[b])         (v-major, DMA'd out + attn rhs)
  - rownorm from v-major tiles (DVE square-reduce)
  - prob partial sums in [v, c] layout, scaled by 1/rownorm, pre-divided by 32
  - AllReduce(prob partial) -> sinkhorn -> concrete-bernoulli mask (all cores)
  - masked softmax over scores -> A -> PE-transpose -> attn matmul accumulated
    across local batches in one PSUM bank; per-core partial summed on host.
"""

import numpy as np

import concourse.bass as bass
import concourse.bacc as bacc
import concourse.tile as tile
from concourse import mybir
from concourse.bass_utils import run_bass_kernel_spmd
from concourse.masks import make_identity

BS, S, V, D, C = 32, 1024, 1024, 512, 64
N_CORES = 8
BL = BS // N_CORES            # batches per core
EPS_SINK = 0.05
SCALE = 1.0 / float(np.sqrt(np.float32(D)))
F32 = mybir.dt.float32
F32R = mybir.dt.float32r
AF = mybir.ActivationFunctionType
AX = mybir.AxisListType

# which matmul groups run in fp32r (4x faster than fp32 on the PE)
USE_F32R_MAIN = False    # x_embT matmul (N=512)
USE_F32R_SCORES = False  # scores matmul (N=512)
USE_F32R_ATTN = False    # attention matmul (N=512)
USE_F32R_PROB = False    # prob matmul (N=64; no speed win from f32r)


def _r(ap, on):
    return ap.bitcast(F32R) if on else ap


def _build_program(stage=3):
    nc = bacc.Bacc("TRN2", target_bir_lowering=False, debug=False,
                   num_devices=N_CORES)

    xs = nc.dram_tensor("xs", [BL, S, V], F32, kind="ExternalInput").ap()
    wt = nc.dram_tensor("wt", [S, D], F32, kind="ExternalInput").ap()
    cet = nc.dram_tensor("cet", [D, C], F32, kind="ExternalInput").ap()
    cnt = nc.dram_tensor("cnt", [D, C], F32, kind="ExternalInput").ap()
    bias = nc.dram_tensor("bias", [128, 4], F32, kind="ExternalInput").ap()
    lnz = nc.dram_tensor("lnz", [V, C], F32, kind="ExternalInput").ap()

    xemb_o = nc.dram_tensor("xemb", [BL, V, D], F32, kind="ExternalOutput").ap()
    pavg_o = nc.dram_tensor("pavg", [V, C], F32, kind="ExternalOutput").ap()
    mask_o = nc.dram_tensor("mask", [V, C], F32, kind="ExternalOutput").ap()
    attn_o = nc.dram_tensor("attnp", [C, D], F32, kind="ExternalOutput").ap()

    with tile.TileContext(nc) as tc:
        with (
            tc.tile_pool(name="consts", bufs=1) as consts,
            tc.tile_pool(name="resid", bufs=1) as resid,
            tc.tile_pool(name="xembT", bufs=2) as xembT_pool,
            tc.tile_pool(name="xload", bufs=3) as x_pool,
            tc.tile_pool(name="work", bufs=2) as work,
            tc.tile_pool(name="small", bufs=4) as small,
            tc.tile_pool(name="psA", bufs=4, space="PSUM") as psA,
            tc.tile_pool(name="psB", bufs=2, space="PSUM") as psB,
            tc.tile_pool(name="psC", bufs=2, space="PSUM") as psC,
            tc.tile_pool(name="dram", bufs=1, space="DRAM") as dram,
        ):
            # ---- constants ----
            wt_sb = consts.tile([128, 8, D], F32)
            nc.sync.dma_start(wt_sb, wt.rearrange("(n p) d -> p n d", p=128))
            cet_sb = consts.tile([128, 4, C], F32)
            nc.sync.dma_start(cet_sb, cet.rearrange("(n p) c -> p n c", p=128))
            cnt_sb = consts.tile([128, 4, C], F32)
            nc.sync.dma_start(cnt_sb, cnt.rearrange("(n p) c -> p n c", p=128))
            bias_sb = consts.tile([128, 4], F32)
            nc.sync.dma_start(bias_sb, bias)
            lnz_sb = consts.tile([128, 8, C], F32)
            nc.sync.dma_start(lnz_sb, lnz.rearrange("(n p) c -> p n c", p=128))
            ident = consts.tile([128, 128], F32)
            make_identity(nc, ident)
            c_eps = consts.tile([128, 1], F32)
            nc.vector.memset(c_eps, 1e-10)
            c_1eps = consts.tile([128, 1], F32)
            nc.vector.memset(c_1eps, 1.0 + 1e-10)

            # ---- residents ----
            xemb_sb = resid.tile([128, BL, 8, D], F32)     # v-major x_emb
            scores_sb = resid.tile([C, BL, V], F32)        # raw scores, c-major
            prob_acc = resid.tile([128, 8, C], F32)        # local prob sum [v,c]

            # ================= phase 1: per-batch embedding =================
            for b in range(BL):
                xembT = xembT_pool.tile([128, 4, V], F32, tag="xembT")
                # main matmul: x_embT[b] = W @ x[b]  (+bias on eviction)
                for half in range(2):
                    ps = [psA.tile([128, 512], F32, tag="mm",
                                   name=f"mmps_{b}_{half}_{i}")
                          for i in range(4)]
                    for sc in range(8):
                        xt = x_pool.tile([128, 512], F32, tag="x")
                        nc.sync.dma_start(
                            xt, xs[b, sc * 128:(sc + 1) * 128,
                                   half * 512:(half + 1) * 512])
                        for dt in range(4):
                            nc.tensor.matmul(
                                ps[dt],
                                _r(wt_sb[:, sc, dt * 128:(dt + 1) * 128],
                                   USE_F32R_MAIN),
                                _r(xt, USE_F32R_MAIN),
                                start=(sc == 0), stop=(sc == 7))
                    for dt in range(4):
                        nc.scalar.activation(
                            xembT[:, dt, half * 512:(half + 1) * 512], ps[dt],
                            AF.Identity, bias=bias_sb[:, dt:dt + 1], scale=1.0)

                # scores[b] = ce @ x_embT[b]   -> [64, 1024]
                for vc in range(2):
                    ps_s = psC.tile([C, 512], F32, tag="sc")
                    for dc in range(4):
                        nc.tensor.matmul(
                            ps_s,
                            _r(cet_sb[:, dc, :], USE_F32R_SCORES),
                            _r(xembT[:, dc, vc * 512:(vc + 1) * 512],
                               USE_F32R_SCORES),
                            start=(dc == 0), stop=(dc == 3))
                    nc.vector.tensor_copy(
                        scores_sb[:, b, vc * 512:(vc + 1) * 512], ps_s)

                # transpose to v-major x_emb (also the x_emb output)
                for vt in range(8):
                    for dt in range(4):
                        ps_t = psB.tile([128, 128], F32, tag="tr")
                        nc.tensor.transpose(
                            ps_t, xembT[:, dt, vt * 128:(vt + 1) * 128], ident)
                        nc.scalar.copy(
                            xemb_sb[:, b, vt, dt * 128:(dt + 1) * 128], ps_t)
                nc.sync.dma_start(
                    xemb_o[b].rearrange("(n p) d -> p n d", p=128),
                    xemb_sb[:, b])

                # row norms (over d) from v-major tiles
                rn = small.tile([128, 8], F32, tag="rn")
                for vt in range(8):
                    scr = work.tile([128, 512], F32, tag="sq")
                    nc.scalar.activation(scr, xemb_sb[:, b, vt], AF.Square,
                                         accum_out=rn[:, vt:vt + 1])
                rns = small.tile([128, 8], F32, tag="rns")
                nc.scalar.activation(rns, rn, AF.Sqrt)
                rni = small.tile([128, 8], F32, tag="rni")
                nc.vector.reciprocal(rni, rns)

                # prob partial: [v, c] = (x_emb . cn^T/32) / rownorm
                for vt in range(8):
                    ps_p = psC.tile([128, C], F32, tag="sc")
                    for dc in range(4):
                        nc.tensor.matmul(
                            ps_p,
                            _r(xembT[:, dc, vt * 128:(vt + 1) * 128],
                               USE_F32R_PROB),
                            _r(cnt_sb[:, dc, :], USE_F32R_PROB),
                            start=(dc == 0), stop=(dc == 3))
                    if b == 0:
                        nc.vector.tensor_scalar_mul(
                            prob_acc[:, vt], ps_p, rni[:, vt:vt + 1])
                    else:
                        pt = work.tile([128, C], F32, tag="pt")
                        nc.vector.tensor_scalar_mul(pt, ps_p, rni[:, vt:vt + 1])
                        nc.vector.tensor_add(
                            prob_acc[:, vt], prob_acc[:, vt], pt)

            # ================= phase 2: allreduce + sinkhorn + mask =========
            if stage >= 2:
                pr_in = dram.tile([V, C], F32)
                pr_out = dram.tile([V, C], F32)
                nc.sync.dma_start(
                    pr_in.rearrange("(n p) c -> p n c", p=128), prob_acc)
                nc.gpsimd.collective_compute(
                    "AllReduce", mybir.AluOpType.add,
                    replica_groups=[list(range(N_CORES))],
                    ins=[pr_in.opt()], outs=[pr_out.opt()])
                pavg_raw = work.tile([128, 8, C], F32, tag="pv", bufs=1)
                nc.sync.dma_start(
                    pavg_raw, pr_out.rearrange("(n p) c -> p n c", p=128))

                q = work.tile([128, 8, C], F32, tag="q", bufs=1)
                nc.scalar.activation(q, pavg_raw, AF.Exp, scale=1.0 / EPS_SINK)
                qs = small.tile([128, 8], F32, tag="qs")
                nc.vector.tensor_reduce(qs, q, axis=AX.X,
                                        op=mybir.AluOpType.add)
                qsi = small.tile([128, 8], F32, tag="qsi")
                nc.vector.reciprocal(qsi, qs)
                pavg = resid.tile([128, 8, C], F32)
                for vt in range(8):
                    nc.vector.tensor_scalar_mul(pavg[:, vt], q[:, vt],
                                                qsi[:, vt:vt + 1])
                nc.sync.dma_start(
                    pavg_o.rearrange("(n p) c -> p n c", p=128), pavg)

                t1 = work.tile([128, 8, C], F32, tag="t1", bufs=1)
                nc.scalar.activation(t1, pavg, AF.Ln, bias=c_eps[:, 0:1],
                                     scale=1.0)
                t2 = work.tile([128, 8, C], F32, tag="t2", bufs=1)
                nc.scalar.activation(t2, pavg, AF.Ln, bias=c_1eps[:, 0:1],
                                     scale=-1.0)
                lg = work.tile([128, 8, C], F32, tag="lg", bufs=1)
                nc.vector.tensor_sub(lg, t1, t2)
                nc.vector.tensor_add(lg, lg, lnz_sb)
                msk = resid.tile([128, 8, C], F32)
                nc.scalar.activation(msk, lg, AF.Sigmoid)
                nc.sync.dma_start(
                    mask_o.rearrange("(n p) c -> p n c", p=128), msk)

                mskT = resid.tile([C, V], F32)
                for vt in range(8):
                    ps_m = psC.tile([C, 128], F32, tag="sc")
                    nc.tensor.transpose(ps_m, msk[:, vt], ident)
                    nc.scalar.copy(mskT[:, vt * 128:(vt + 1) * 128], ps_m)

            # ================= phase 3: masked attention ====================
            if stage >= 3:
                attn_ps = psA.tile([C, D], F32, tag="mm")
                for b in range(BL):
                    sm = work.tile([C, V], F32, tag="sm")
                    nc.vector.tensor_mul(sm, scores_sb[:, b], mskT)
                    mx = small.tile([C, 1], F32, tag="mx")
                    nc.vector.tensor_reduce(mx, sm, axis=AX.X,
                                            op=mybir.AluOpType.max)
                    nmx = small.tile([C, 1], F32, tag="nmx")
                    nc.scalar.mul(nmx, mx, -SCALE)
                    nc.scalar.activation(sm, sm, AF.Exp, bias=nmx[:, 0:1],
                                         scale=SCALE)
                    es = small.tile([C, 1], F32, tag="es")
                    nc.vector.tensor_reduce(es, sm, axis=AX.X,
                                            op=mybir.AluOpType.add)
                    esi = small.tile([C, 1], F32, tag="esi")
                    nc.vector.reciprocal(esi, es)
                    esi32 = small.tile([C, 1], F32, tag="esi32")
                    nc.scalar.mul(esi32, esi, 1.0 / BS)
                    a = sm
                    nc.vector.tensor_scalar_mul(a, sm, esi32[:, 0:1])

                    at = work.tile([128, 8, C], F32, tag="at")
                    for vt in range(8):
                        ps_at = psB.tile([128, C], F32, tag="tr")
                        nc.tensor.transpose(
                            ps_at, a[:, vt * 128:(vt + 1) * 128],
                            ident[:C, :C])
                        nc.scalar.copy(at[:, vt], ps_at)
                    for vt in range(8):
                        nc.tensor.matmul(
                            attn_ps,
                            _r(at[:, vt], USE_F32R_ATTN),
                            _r(xemb_sb[:, b, vt], USE_F32R_ATTN),
                            start=(b == 0 and vt == 0),
                            stop=(b == BL - 1 and vt == 7))
                fin = work.tile([C, D], F32, tag="fin")
                nc.vector.tensor_copy(fin, attn_ps)
                nc.sync.dma_start(attn_o, fin)

    nc.compile()
    return nc


_PROGRAM = None


def _get_program():
    global _PROGRAM
    if _PROGRAM is None:
        _PROGRAM = _build_program()
    return _PROGRAM


def _make_in_maps(x, cluster_emb, W, b, noise):
    x = np.ascontiguousarray(x, dtype=np.float32)
    W = np.asarray(W, dtype=np.float32)
    ce = np.asarray(cluster_emb, dtype=np.float32)
    bb = np.asarray(b, dtype=np.float32)
    noise = np.asarray(noise, dtype=np.float32)

    wt = np.ascontiguousarray(W.T)                      # (S, D)
    cet = np.ascontiguousarray(ce.T)                    # (D, C)
    n = np.sqrt((ce * ce).sum(axis=1, keepdims=True))
    cn = ce / np.maximum(n, 1e-12)
    cnt = np.ascontiguousarray(cn.T / np.float32(BS))   # (D, C), pre-scaled
    bias = np.ascontiguousarray(bb.reshape(4, 128).T)   # (128, 4)
    lnz = (np.log(noise) - np.log(1.0 - noise)).astype(np.float32)

    in_maps = []
    for i in range(N_CORES):
        in_maps.append({
            "xs": x[i * BL:(i + 1) * BL],
            "wt": wt, "cet": cet, "cnt": cnt, "bias": bias, "lnz": lnz,
        })
    return in_maps


def kernel(x, cluster_emb, W, b, noise):
    nc = _get_program()
    in_maps = _make_in_maps(x, cluster_emb, W, b, noise)
    res = run_bass_kernel_spmd(nc, in_maps, list(range(N_CORES))).results

    x_emb = np.concatenate([res[i]["xemb"] for i in range(N_CORES)], axis=0)
    prob_avg = res[0]["pavg"]
    mask = res[0]["mask"]
    cluster_out = np.sum([res[i]["attnp"] for i in range(N_CORES)], axis=0,
                         dtype=np.float32)
    prob_pb = np.broadcast_to(prob_avg, (BS, V, C))
    mask_pb = np.broadcast_to(mask, (BS, V, C))
    return (prob_avg, cluster_out, prob_pb, mask_pb, x_emb)


# revision 34
# speedup vs baseline: 173788.3733x; 173788.3733x over previous
"""Trainium2 Bass kernel for nn_Cluster_assigner (moe_routing).

Data-parallel over batch across 8 NeuronCores (4 batches/core):
  - x_embT[b] = W @ x[b] + bias  (d-major) via PE matmul
  - scores[b] = cluster_emb @ x_embT[b]         (c-major, raw attention scores)
  - x_emb[b]  = PE-transpose(x_embT[b])         (v-major, DMA'd out + attn rhs)
  - rownorm from v-major tiles (ACT square+accum)
  - prob partial sums in [v, c] layout, scaled by 1/rownorm, pre-divided by 32
  - AllReduce(prob partial) -> sinkhorn -> concrete-bernoulli mask (all cores)
  - masked softmax over scores -> A -> PE-transpose -> attn matmul accumulated
    across local batches in one PSUM bank; per-core partial summed on host.
"""

import numpy as np

import concourse.bass as bass
import concourse.bacc as bacc
import concourse.tile as tile
from concourse import mybir
from concourse.bass_utils import run_bass_kernel_spmd
from concourse.masks import make_identity

BS, S, V, D, C = 32, 1024, 1024, 512, 64
N_CORES = 8
BL = BS // N_CORES            # batches per core
EPS_SINK = 0.05
SCALE = 1.0 / float(np.sqrt(np.float32(D)))
F32 = mybir.dt.float32
F32R = mybir.dt.float32r
AF = mybir.ActivationFunctionType
AX = mybir.AxisListType

# fp32r (TF32-class operand rounding) runs the N=512 matmuls at 4x fp32
# throughput on the PE. Measured end-to-end error ~1e-4 vs ~3e-6 for fp32.
USE_F32R = True


def _build_program(stage=3, reps=1, use_f32r=None):
    mmdt = F32R if (USE_F32R if use_f32r is None else use_f32r) else F32
    nc = bacc.Bacc("TRN2", target_bir_lowering=False, debug=False,
                   num_devices=N_CORES)

    xs = nc.dram_tensor("xs", [BL, S, V], mmdt, kind="ExternalInput").ap()
    wt = nc.dram_tensor("wt", [S, D], mmdt, kind="ExternalInput").ap()
    cet = nc.dram_tensor("cet", [D, C], mmdt, kind="ExternalInput").ap()
    cesc = nc.dram_tensor("cesc", [C, 2], F32, kind="ExternalInput").ap()
    bias = nc.dram_tensor("bias", [128, 4], F32, kind="ExternalInput").ap()
    lnz = nc.dram_tensor("lnz", [V, C], F32, kind="ExternalInput").ap()

    xemb_o = nc.dram_tensor("xemb", [BL, V, D], F32, kind="ExternalOutput").ap()
    pavg_o = nc.dram_tensor("pavg", [V, C], F32, kind="ExternalOutput").ap()
    mask_o = nc.dram_tensor("mask", [V, C], F32, kind="ExternalOutput").ap()
    attn_o = nc.dram_tensor("attnp", [C, D], F32, kind="ExternalOutput").ap()

    with tile.TileContext(nc) as tc:
        with (
            tc.tile_pool(name="consts", bufs=1) as consts,
            tc.tile_pool(name="resid", bufs=1) as resid,
            tc.tile_pool(name="xembT", bufs=2) as xembT_pool,
            tc.tile_pool(name="xload", bufs=3) as x_pool,
            tc.tile_pool(name="work", bufs=2) as work,
            tc.tile_pool(name="small", bufs=4) as small,
            tc.tile_pool(name="psA", bufs=4, space="PSUM") as psA,
            tc.tile_pool(name="psB", bufs=2, space="PSUM") as psB,
            tc.tile_pool(name="psC", bufs=2, space="PSUM") as psC,
            tc.tile_pool(name="dram", bufs=1, space="DRAM") as dram,
        ):
            # ---- constants ----
            wt_sb = consts.tile([128, 8, D], mmdt)
            wt_r = wt.rearrange("(n p) d -> p n d", p=128)
            for sc in range(8):
                nc.scalar.dma_start(wt_sb[:, sc], wt_r[:, sc])
            cet_sb = consts.tile([128, 4, C], mmdt)
            nc.scalar.dma_start(cet_sb, cet.rearrange("(n p) c -> p n c", p=128))
            cesc_sb = consts.tile([C, 2], F32)
            nc.scalar.dma_start(cesc_sb, cesc)
            bias_sb = consts.tile([128, 4], F32)
            nc.scalar.dma_start(bias_sb, bias)
            lnz_sb = consts.tile([128, 8, C], F32)
            nc.scalar.dma_start(lnz_sb, lnz.rearrange("(n p) c -> p n c", p=128))
            ident = consts.tile([128, 128], F32)
            make_identity(nc, ident)
            c_eps = consts.tile([128, 1], F32)
            nc.vector.memset(c_eps, 1e-10)
            c_1eps = consts.tile([128, 1], F32)
            nc.vector.memset(c_1eps, 1.0 + 1e-10)

            # ---- residents ----
            xemb_sb = resid.tile([128, BL, 8, D], mmdt)     # v-major x_emb
            scores_sb = resid.tile([C, BL, V], F32)        # raw scores, c-major
            prob_acc = resid.tile([128, 8, C], F32)        # local prob sum
            nmx_all = resid.tile([C, BL], F32)             # -SCALE*max|scores|

            def main_mm(b):
                """x_embT[b] = W @ x[b] + bias -> returns xembT tile."""
                xembT = xembT_pool.tile([128, 4, V], mmdt, tag="xembT",
                                        name=f"xembT_{b}")
                for half in range(2):
                    ps = [psA.tile([128, 512], F32, tag="mm",
                                   name=f"mmps_{b}_{half}_{i}")
                          for i in range(4)]
                    for q in range(2):
                        xt = x_pool.tile([128, 4, 512], mmdt, tag="x",
                                         name="xt")
                        xs_q = xs[b, q * 512:(q + 1) * 512,
                                  half * 512:(half + 1) * 512].rearrange(
                                      "(n p) v -> p n v", p=128)
                        for si in range(4):
                            nc.sync.dma_start(xt[:, si], xs_q[:, si])
                        for dt in range(4):
                            for si in range(4):
                                sc = q * 4 + si
                                nc.tensor.matmul(
                                    ps[dt],
                                    wt_sb[:, sc, dt * 128:(dt + 1) * 128],
                                    xt[:, si],
                                    start=(sc == 0), stop=(sc == 7))
                    for dt in range(4):
                        nc.scalar.activation(
                            xembT[:, dt, half * 512:(half + 1) * 512], ps[dt],
                            AF.Identity, bias=bias_sb[:, dt:dt + 1], scale=1.0)
                return xembT

            def tail(b, xembT):
                """scores, v-major transpose, rownorm, prob for batch b."""
                # scores[b] = ce @ x_embT[b]   -> [64, 1024]
                for vc in range(2):
                    ps_s = psC.tile([C, 512], F32, tag="sc", name="ps_s")
                    for dc in range(4):
                        nc.tensor.matmul(
                            ps_s,
                            cet_sb[:, dc, :],
                            xembT[:, dc, vc * 512:(vc + 1) * 512],
                            start=(dc == 0), stop=(dc == 3))
                    nc.vector.tensor_scalar_mul(
                        scores_sb[:, b, vc * 512:(vc + 1) * 512], ps_s,
                        cesc_sb[:, 0:1])
                mxb = small.tile([C, 1], F32, tag="mxb", name="mxb")
                nc.vector.tensor_reduce(mxb, scores_sb[:, b], axis=AX.X,
                                        op=mybir.AluOpType.max,
                                        apply_absolute_value=True)
                nc.vector.tensor_mul(mxb, mxb, cesc_sb[:, 1:2])
                nc.scalar.mul(nmx_all[:, b:b + 1], mxb, -SCALE)

                # transpose to v-major x_emb; paired into [128, 256] psum
                for vt in range(8):
                    for dp in range(2):
                        ps_t = psB.tile([128, 256], F32, tag="tr", name="ps_t")
                        for k in range(2):
                            dt = dp * 2 + k
                            nc.tensor.transpose(
                                ps_t[:, k * 128:(k + 1) * 128],
                                xembT[:, dt,
                                      vt * 128:(vt + 1) * 128].bitcast(F32),
                                ident)
                        if (vt + dp) % 2 == 0:
                            nc.vector.tensor_copy(
                                xemb_sb[:, b, vt,
                                        dp * 256:(dp + 1) * 256], ps_t)
                        else:
                            nc.scalar.copy(
                                xemb_sb[:, b, vt,
                                        dp * 256:(dp + 1) * 256], ps_t)
                nc.sync.dma_start(
                    xemb_o[b].rearrange("(n p) d -> p n d", p=128),
                    xemb_sb[:, b].bitcast(F32))

                # row norms over d; alternate engines to balance ACT/DVE
                rn = small.tile([128, 8], F32, tag="rn", name="rn")
                for vt in range(8):
                    scr = work.tile([128, 512], F32, tag="sq", name="scr")
                    if (b + vt) % 2 == 0:
                        nc.scalar.activation(scr,
                                             xemb_sb[:, b, vt].bitcast(F32),
                                             AF.Square,
                                             accum_out=rn[:, vt:vt + 1])
                    else:
                        nc.vector.tensor_mul(scr,
                                             xemb_sb[:, b, vt].bitcast(F32),
                                             xemb_sb[:, b, vt].bitcast(F32))
                        nc.vector.tensor_reduce(rn[:, vt:vt + 1], scr,
                                                axis=AX.X,
                                                op=mybir.AluOpType.add)
                rns = small.tile([128, 8], F32, tag="rns", name="rns")
                nc.scalar.activation(rns, rn, AF.Sqrt)
                rni = small.tile([128, 8], F32, tag="rni", name="rni")
                nc.vector.reciprocal(rni, rns)
                return rni

            def prob_part(b, rni):
                # prob partial [v, c]: transpose of ce-scaled scores; evict to
                # SBUF via ACT (frees PSUM slots fast), then scale+accumulate
                # on DVE off the PE critical path.
                pp = work.tile([128, 8, C], F32, tag="pp", name="pp")
                for vt in range(8):
                    ps_pt = psB.tile([128, C], F32, tag="tr", name="ps_pt")
                    nc.tensor.transpose(
                        ps_pt, scores_sb[:, b, vt * 128:(vt + 1) * 128],
                        ident[:C, :C])
                    if vt % 2 == 0:
                        nc.vector.tensor_copy(pp[:, vt], ps_pt)
                    else:
                        nc.scalar.copy(pp[:, vt], ps_pt)
                for vt in range(8):
                    if b == 0:
                        nc.vector.tensor_scalar_mul(
                            prob_acc[:, vt], pp[:, vt], rni[:, vt:vt + 1])
                    else:
                        pt = work.tile([128, C], F32, tag="pt", name="pt")
                        nc.vector.tensor_scalar_mul(pt, pp[:, vt],
                                                    rni[:, vt:vt + 1])
                        nc.vector.tensor_add(
                            prob_acc[:, vt], prob_acc[:, vt], pt)

            def phase2():
                pr_in = dram.tile([V, C], F32, tag="pr_in", name="pr_in")
                pr_out = dram.tile([V, C], F32, tag="pr_out", name="pr_out")
                nc.sync.dma_start(
                    pr_in.rearrange("(n p) c -> p n c", p=128), prob_acc)
                nc.gpsimd.collective_compute(
                    "AllReduce", mybir.AluOpType.add,
                    replica_groups=[list(range(N_CORES))],
                    ins=[pr_in.opt()], outs=[pr_out.opt()])
                pv = work.tile([128, 8, C], F32, tag="pv", bufs=1, name="pv")
                nc.sync.dma_start(
                    pv, pr_out.rearrange("(n p) c -> p n c", p=128))

                # sinkhorn: pv -> exp -> row-normalize (in place)
                nc.scalar.activation(pv, pv, AF.Exp, scale=1.0 / EPS_SINK)
                qs = small.tile([128, 8], F32, tag="qs", name="qs")
                nc.vector.tensor_reduce(qs, pv, axis=AX.X,
                                        op=mybir.AluOpType.add)
                qsi = small.tile([128, 8], F32, tag="qsi", name="qsi")
                nc.vector.reciprocal(qsi, qs)
                for vt in range(8):
                    nc.vector.tensor_scalar_mul(pv[:, vt], pv[:, vt],
                                                qsi[:, vt:vt + 1])
                nc.scalar.dma_start(
                    pavg_o.rearrange("(n p) c -> p n c", p=128), pv)

                t1 = work.tile([128, 8, C], F32, tag="t1", bufs=1, name="t1")
                nc.scalar.activation(t1, pv, AF.Ln, bias=c_eps[:, 0:1],
                                     scale=1.0)
                t2 = work.tile([128, 8, C], F32, tag="t2", bufs=1, name="t2")
                nc.scalar.activation(t2, pv, AF.Ln, bias=c_1eps[:, 0:1],
                                     scale=-1.0)
                nc.vector.tensor_sub(t1, t1, t2)
                nc.vector.tensor_add(t1, t1, lnz_sb)
                msk = t2
                nc.scalar.activation(msk, t1, AF.Sigmoid)
                nc.scalar.dma_start(
                    mask_o.rearrange("(n p) c -> p n c", p=128), msk)

                mskT = work.tile([C, V], F32, tag="mskT", bufs=1, name="mskT")
                for vt in range(8):
                    ps_m = psC.tile([C, 128], F32, tag="sc", name="ps_m")
                    nc.tensor.transpose(ps_m, msk[:, vt], ident)
                    nc.scalar.copy(mskT[:, vt * 128:(vt + 1) * 128], ps_m)
                return mskT

            def phase3(mskT):
                attn_acc = work.tile([C, D], F32, tag="attacc", bufs=1,
                                     name="attn_acc")

                def softmax_t(b):
                    # e = exp(SCALE*(scores*ce32*mask - max|scores|))
                    sm = work.tile([C, V], F32, tag="sm", name="sm")
                    nc.vector.scalar_tensor_tensor(
                        sm, scores_sb[:, b], cesc_sb[:, 1:2], mskT,
                        op0=mybir.AluOpType.mult, op1=mybir.AluOpType.mult)
                    es = small.tile([C, 1], F32, tag="es", name="es")
                    nc.scalar.activation(sm, sm, AF.Exp,
                                         bias=nmx_all[:, b:b + 1],
                                         scale=SCALE, accum_out=es)
                    esi32 = small.tile([C, 1], F32, tag="esi32", name="esi32")
                    nc.vector.reciprocal(esi32, es)
                    nc.scalar.mul(esi32, esi32, 1.0 / BS)
                    at = work.tile([128, 8, C], mmdt, tag="at", name="at")
                    for vt in range(8):
                        ps_at = psB.tile([128, C], F32, tag="tr", name="ps_at")
                        nc.tensor.transpose(
                            ps_at, sm[:, vt * 128:(vt + 1) * 128],
                            ident[:C, :C])
                        nc.scalar.copy(at[:, vt], ps_at)
                    return at, esi32

                def attn_mm(b, at, esi32):
                    psb = psA.tile([C, D], F32, tag="mm", name=f"attnps{b}")
                    for vt in range(8):
                        nc.tensor.matmul(
                            psb, at[:, vt], xemb_sb[:, b, vt],
                            start=(vt == 0), stop=(vt == 7))
                    if b == 0:
                        nc.scalar.activation(attn_acc, psb, AF.Copy,
                                             scale=esi32[:, 0:1])
                    else:
                        ev = work.tile([C, D], F32, tag="attev", name="ev")
                        nc.scalar.activation(ev, psb, AF.Copy,
                                             scale=esi32[:, 0:1])
                        nc.vector.tensor_add(attn_acc, attn_acc, ev)
                    return None

                prev = softmax_t(0)
                for b in range(1, BL):
                    cur = softmax_t(b)
                    attn_mm(b - 1, *prev)
                    prev = cur
                attn_mm(BL - 1, *prev)
                nc.scalar.dma_start(attn_o, attn_acc)

            for rep in range(reps):
                stages = []          # (b, xembT) awaiting tail
                probs = []           # (b, rni) awaiting prob_part
                for b in range(BL):
                    xembT_cur = main_mm(b)
                    if stages:
                        pb, pxembT = stages.pop(0)
                        probs.append((pb, tail(pb, pxembT)))
                    if len(probs) > 1:
                        qb, qrni = probs.pop(0)
                        prob_part(qb, qrni)
                    stages.append((b, xembT_cur))
                while stages:
                    pb, pxembT = stages.pop(0)
                    probs.append((pb, tail(pb, pxembT)))
                while probs:
                    qb, qrni = probs.pop(0)
                    prob_part(qb, qrni)
                if stage >= 2:
                    mskT = phase2()
                if stage >= 3:
                    phase3(mskT)

    nc.compile()
    return nc


_PROGRAM = None


def _get_program():
    global _PROGRAM
    if _PROGRAM is None:
        _PROGRAM = _build_program()
    return _PROGRAM


def _make_in_maps(x, cluster_emb, W, b, noise):
    x = np.ascontiguousarray(x, dtype=np.float32)
    W = np.asarray(W, dtype=np.float32)
    ce = np.asarray(cluster_emb, dtype=np.float32)
    bb = np.asarray(b, dtype=np.float32)
    noise = np.asarray(noise, dtype=np.float32)

    wt = np.ascontiguousarray(W.T)                      # (S, D)
    cet = np.ascontiguousarray(ce.T)                    # (D, C)
    n = np.maximum(np.sqrt((ce * ce).sum(axis=1, keepdims=True)), 1e-12)
    cesc = np.ascontiguousarray(
        np.concatenate([1.0 / (np.float32(BS) * n), np.float32(BS) * n],
                       axis=1).astype(np.float32))      # (C, 2)
    bias = np.ascontiguousarray(bb.reshape(4, 128).T)   # (128, 4)
    lnz = (np.log(noise) - np.log(1.0 - noise)).astype(np.float32)

    in_maps = []
    for i in range(N_CORES):
        in_maps.append({
            "xs": x[i * BL:(i + 1) * BL],
            "wt": wt, "cet": cet, "cesc": cesc, "bias": bias, "lnz": lnz,
        })
    return in_maps


def kernel(x, cluster_emb, W, b, noise):
    nc = _get_program()
    in_maps = _make_in_maps(x, cluster_emb, W, b, noise)
    res = run_bass_kernel_spmd(nc, in_maps, list(range(N_CORES))).results

    x_emb = np.concatenate([res[i]["xemb"] for i in range(N_CORES)], axis=0)
    prob_avg = res[0]["pavg"]
    mask = res[0]["mask"]
    cluster_out = np.sum([res[i]["attnp"] for i in range(N_CORES)], axis=0,
                         dtype=np.float32)
    prob_pb = np.broadcast_to(prob_avg, (BS, V, C))
    mask_pb = np.broadcast_to(mask, (BS, V, C))
    return (prob_avg, cluster_out, prob_pb, mask_pb, x_emb)
